# revision 13
# baseline (speedup 1.0000x reference)
"""ResNet34 + LSTM decoder on 8 TRN2 NeuronCores (Bass/Tile).

Sharding: data-parallel over batch (2 images/core) for the ResNet with
cross-core AllGather of BN partial stats (training-mode BatchNorm needs
full-batch statistics); fc1/gx sharded over output dim; LSTM sharded over
the 4H gate dim with a per-step AllGather of h^T; heads replicated.
All rank-dependent selection is done host-side via per-core inputs.
"""
import numpy as np
import ml_dtypes

N_CORES = 8
B = 16
BPC = B // N_CORES
HID = 1024
DEC = 196
ACTN = 197
EPS = 1e-5
CFG = [(64, 3, 1), (128, 4, 2), (256, 6, 2), (512, 3, 2)]
RES = {1: 56, 2: 28, 3: 14, 4: 7}
CT_OF = {1: 1, 2: 1, 3: 2, 4: 4}

_cached = {}


def _ceil(a, b):
    return (a + b - 1) // b


# ---------------------------------------------------------------- host prep

def _conv_lhsT(w):
    Co, Ci, kh, kw = w.shape
    KT, MT = _ceil(Ci, 128), _ceil(Co, 128)
    out = np.zeros((kh * kw, KT, MT, 128, 128), np.float32)
    for t in range(kh * kw):
        ky, kx = t // kw, t % kw
        for kt in range(KT):
            for mt in range(MT):
                blk = w[mt * 128:mt * 128 + 128, kt * 128:kt * 128 + 128, ky, kx]
                out[t, kt, mt, :blk.shape[1], :blk.shape[0]] = blk.T
    return out.astype(ml_dtypes.bfloat16)


def _gb_pack(g, b):
    C = g.shape[0]
    MT = _ceil(C, 128)
    out = np.zeros((128, 2, MT), np.float32)
    for mt in range(MT):
        n = min(128, C - mt * 128)
        out[:n, 0, mt] = g[mt * 128:mt * 128 + n]
        out[:n, 1, mt] = b[mt * 128:mt * 128 + n]
    return out


def _host_prep(inputs):
    P = inputs
    x = np.asarray(P["x"], np.float32)
    rp = P["resnet_params"]
    convs = []

    def add_conv(name, w, g, b, stride, Hin, Hout):
        w = np.asarray(w, np.float32)
        Co, Ci, kh, kw = w.shape
        convs.append(dict(
            name=name, stride=stride, Hin=Hin, Hout=Hout, kh=kh, kw=kw,
            KT=_ceil(Ci, 128), MT=_ceil(Co, 128), T=kh * kw,
            wq=_conv_lhsT(w),
            gb=_gb_pack(np.asarray(g, np.float32), np.asarray(b, np.float32))))

    in_res = 56
    blocks_meta = []
    for li, ((oc, nb, st), blocks) in enumerate(zip(CFG, rp["layers"]), start=1):
        R = RES[li]
        for bi, bp in enumerate(blocks):
            s = st if bi == 0 else 1
            rin = in_res if bi == 0 else R
            add_conv(f"l{li}b{bi}c1", bp["w1"], bp["g1"], bp["b1"], s, rin, R)
            add_conv(f"l{li}b{bi}c2", bp["w2"], bp["g2"], bp["b2"], 1, R, R)
            if "wd" in bp:
                add_conv(f"l{li}b{bi}ds", bp["wd"], bp["gd"], bp["bd"], s, rin, R)
            blocks_meta.append((li, bi, "wd" in bp))
        in_res = R

    # stem lhsT: partition p = isx*12 + (py*2+px)*3 + c ; mm group syi
    wstem = np.asarray(rp["stem"]["w"], np.float32)
    stem_lhsT = np.zeros((4, 48, 64), np.float32)
    for syi, sy in enumerate(range(-2, 2)):
        for isx in range(4):
            sx = isx - 2
            for py in range(2):
                ky = 2 * sy + 3 + py
                if not (0 <= ky <= 6):
                    continue
                for px in range(2):
                    kx = 2 * sx + 3 + px
                    if not (0 <= kx <= 6):
                        continue
                    for c in range(3):
                        stem_lhsT[syi, isx * 12 + (py * 2 + px) * 3 + c, :] = wstem[:, c, ky, kx]
    stem_gb = _gb_pack(np.asarray(rp["stem"]["g"], np.float32),
                       np.asarray(rp["stem"]["b"], np.float32))

    w1 = np.asarray(P["w1"], np.float32)
    b1 = np.asarray(P["b1"], np.float32)
    w_ih = np.asarray(P["w_ih"], np.float32)
    w_hh = np.asarray(P["w_hh"], np.float32)
    bihh = np.asarray(P["b_ih"], np.float32) + np.asarray(P["b_hh"], np.float32)
    whcat = np.concatenate([np.asarray(P["wh1"], np.float32),
                            np.asarray(P["wh2"], np.float32)], 0)
    bhcat = np.concatenate([np.asarray(P["bh1"], np.float32),
                            np.asarray(P["bh2"], np.float32)], 0)

    in_maps = []
    for c in range(N_CORES):
        xs = x[c * BPC:(c + 1) * BPC]
        xph = np.stack([xs[:, :, py::2, px::2] for py in range(2) for px in range(2)], axis=1)
        m = {"x": np.ascontiguousarray(xph)}  # [BPC, 4, 3, 112, 112]
        for cv in convs:
            m[f"w_{cv['name']}"] = cv["wq"]
            m[f"gb_{cv['name']}"] = cv["gb"]
        m["w_stem"] = stem_lhsT
        m["gb_stem"] = stem_gb
        u = slice(128 * c, 128 * (c + 1))
        w1s = w1[u, :]
        m["w1sT"] = np.stack([np.ascontiguousarray(w1s[:, 128 * t:128 * (t + 1)].T)
                              for t in range(196)]).astype(ml_dtypes.bfloat16)
        m["b1row"] = b1[u].reshape(1, 128).astype(ml_dtypes.bfloat16)
        rows = np.concatenate([np.arange(128 * c, 128 * (c + 1)) + k * HID for k in (0, 1, 3, 2)])
        m["wihT"] = np.stack([np.ascontiguousarray(w_ih[rows][:, 128 * k:128 * (k + 1)].T)
                              for k in range(8)]).astype(np.float32)
        m["whhT"] = np.stack([np.ascontiguousarray(w_hh[rows][:, 128 * k:128 * (k + 1)].T)
                              for k in range(8)]).astype(np.float32)
        m["brow"] = bihh[rows].reshape(1, 512).astype(np.float32)
        m["whcT"] = np.stack([np.ascontiguousarray(whcat[:, 128 * k:128 * (k + 1)].T)
                              for k in range(8)]).astype(ml_dtypes.bfloat16)
        m["bhrow"] = bhcat.reshape(1, 198).astype(ml_dtypes.bfloat16)
        m["ident"] = np.eye(16, dtype=np.float32)
        m["ones_bf"] = np.ones((1, 128), ml_dtypes.bfloat16)
        m["ones_f"] = np.ones((1, 16), np.float32)
        in_maps.append(m)
    return convs, blocks_meta, in_maps


# ---------------------------------------------------------------- builder

def _build(convs, blocks_meta):
    import concourse.mybir as mybir
    import concourse.tile as tile
    from concourse import bacc

    F32 = mybir.dt.float32
    BF16 = mybir.dt.bfloat16
    AF = mybir.ActivationFunctionType
    OP = mybir.AluOpType
    AX = mybir.AxisListType
    RG = [list(range(N_CORES))]

    nc = bacc.Bacc("TRN2", target_bir_lowering=False, debug=False, num_devices=N_CORES)
    cvmeta = {cv["name"]: cv for cv in convs}

    d_in = {"x": nc.dram_tensor("x", [BPC, 4, 3, 112, 112], F32, kind="ExternalInput")}
    for cv in convs:
        n = cv["name"]
        d_in[f"w_{n}"] = nc.dram_tensor(f"w_{n}", list(cv["wq"].shape), BF16, kind="ExternalInput")
        d_in[f"gb_{n}"] = nc.dram_tensor(f"gb_{n}", [128, 2, cv["MT"]], F32, kind="ExternalInput")
    for nm, shp, dt in [("w_stem", [4, 48, 64], F32), ("gb_stem", [128, 2, 1], F32),
                        ("w1sT", [196, 128, 128], BF16), ("b1row", [1, 128], BF16),
                        ("wihT", [8, 128, 512], F32), ("whhT", [8, 128, 512], F32),
                        ("brow", [1, 512], F32), ("whcT", [8, 128, 198], BF16),
                        ("bhrow", [1, 198], BF16), ("ident", [16, 16], F32),
                        ("ones_bf", [1, 128], BF16), ("ones_f", [1, 16], F32)]:
        d_in[nm] = nc.dram_tensor(nm, shp, dt, kind="ExternalInput")

    o_table = nc.dram_tensor("table", [B * DEC, ACTN], F32, kind="ExternalOutput")
    o_values = nc.dram_tensor("values", [B * DEC, 1], F32, kind="ExternalOutput")
    o_feat = nc.dram_tensor("dbg_feat", [BPC, 25088], F32, kind="ExternalOutput")
    o_xin = nc.dram_tensor("dbg_xin", [16, 128], F32, kind="ExternalOutput")
    o_gx = nc.dram_tensor("dbg_gx", [16, 512], F32, kind="ExternalOutput")
    o_hT = nc.dram_tensor("dbg_hT", [128, 128], F32, kind="ExternalOutput")

    with tile.TileContext(nc) as tc:
        padA, padB = {}, {}
        for s in (1, 2, 3, 4):
            H, CT = RES[s], CT_OF[s]
            padA[s] = nc.alloc_sbuf_tensor(f"pA{s}", [128, CT, 2, H + 2, H + 2], BF16)
            padB[s] = nc.alloc_sbuf_tensor(f"pB{s}", [128, CT, 2, H + 2, H + 2], BF16)
            nc.gpsimd.memset(padA[s][:, :, :, :, :], 0.0)
            nc.gpsimd.memset(padB[s][:, :, :, :, :], 0.0)
        # shared raw buffers (bytes of largest stage use)
        rawA = nc.alloc_sbuf_tensor("rawA", [128, 6272], BF16)
        rawB = nc.alloc_sbuf_tensor("rawB", [128, 6272], BF16)
        rawD = nc.alloc_sbuf_tensor("rawD", [128, 1568], BF16)
        epsT = nc.alloc_sbuf_tensor("epsT", [128, 1], F32)
        nc.vector.memset(epsT[:, :], EPS)

        def raw_view(buf, s):
            H, CT = RES[s], CT_OF[s]
            return buf[:, 0:CT * 2 * H * H].rearrange(
                "p (m i h w) -> p m i h w", m=CT, i=2, h=H)

        with tc.tile_pool(name="gbp", bufs=3) as gbp, \
             tc.tile_pool(name="statp", bufs=3) as statp, \
             tc.tile_pool(name="abp", bufs=4) as abp, \
             tc.tile_pool(name="smallp", bufs=3) as smallp, \
             tc.tile_pool(name="dram", bufs=8, space="DRAM") as dram:

            def bn_reduce(stats_sb, nstats, gb_sb, MT, Cpart):
                aggr = smallp.tile([128, MT, 2], F32, tag="aggr")
                nc.vector.memset(aggr[:, :, :], 0.0)
                for mt in range(MT):
                    nc.vector.bn_aggr(aggr[0:Cpart, mt, :], stats_sb[0:Cpart, mt, :, :])
                m = aggr[:, :, 0]
                vvar = aggr[:, :, 1]
                agb = smallp.tile([128, 2 * MT], F32, tag="agb")
                nc.vector.tensor_copy(agb[:, 0:MT], m)
                sq = smallp.tile([128, MT], F32, tag="sq")
                nc.vector.tensor_tensor(sq[:, :], m, m, OP.mult)
                nc.vector.tensor_tensor(agb[:, MT:2 * MT], vvar, sq[:, :], OP.add)
                bin_ = dram.tile([128, 2 * MT], F32, tag="bnin")
                bout = dram.tile([8, 128, 2 * MT], F32, tag="bnout")
                nc.sync.dma_start(bin_[:, :], agb[:, :])
                nc.gpsimd.collective_compute("AllGather", OP.bypass, ins=[bin_.opt()],
                                             outs=[bout.opt()], replica_groups=RG)
                red_in = smallp.tile([128, 2 * MT, 8], F32, tag="redin")
                nc.sync.dma_start(red_in[:, :, :], bout[:, :, :].rearrange("r p s -> p s r"))
                red = smallp.tile([128, 2 * MT], F32, tag="red")
                nc.vector.tensor_reduce(red[:, :], red_in[:, :, :], AX.X, OP.add)
                meanf = smallp.tile([128, MT], F32, tag="meanf")
                varf = smallp.tile([128, MT], F32, tag="varf")
                nc.vector.tensor_scalar_mul(meanf[:, :], red[:, 0:MT], 1.0 / N_CORES)
                nc.vector.tensor_scalar_mul(varf[:, :], red[:, MT:2 * MT], 1.0 / N_CORES)
                msq = smallp.tile([128, MT], F32, tag="msq")
                nc.vector.tensor_tensor(msq[:, :], meanf[:, :], meanf[:, :], OP.mult)
                nc.vector.tensor_tensor(varf[:, :], varf[:, :], msq[:, :], OP.subtract)
                std = smallp.tile([128, MT], F32, tag="std")
                nc.scalar.activation(std[:, :], varf[:, :], AF.Sqrt, bias=epsT[:, :])
                rstd = smallp.tile([128, MT], F32, tag="rstd")
                nc.vector.reciprocal(rstd[:, :], std[:, :])
                ab = abp.tile([128, 2, MT], F32, tag="ab")
                nc.vector.tensor_tensor(ab[:, 0, :], gb_sb[:, 0, :], rstd[:, :], OP.mult)
                bt = smallp.tile([128, MT], F32, tag="bt")
                nc.vector.tensor_tensor(bt[:, :], ab[:, 0, :], meanf[:, :], OP.mult)
                nc.vector.tensor_tensor(ab[:, 1, :], gb_sb[:, 1, :], bt[:, :], OP.subtract)
                return ab

            def conv(cvn, inbuf, rawbuf, wpool, psump):
                cv = cvmeta[cvn]
                T, KT, MT, st = cv["T"], cv["KT"], cv["MT"], cv["stride"]
                Ho, kw = cv["Hout"], cv["kw"]
                po = 1 if cv["kh"] == 1 else 0  # pad offset for 1x1 (pad-0) convs
                wsb = wpool.tile([128, T * KT * MT, 128], BF16, tag="convw")
                nc.sync.dma_start(wsb[:, :, :],
                                  d_in[f"w_{cvn}"][:, :, :, :, :].rearrange("t k m c o -> c (t k m) o"))
                gb_sb = gbp.tile([128, 2, MT], F32, tag="gb")
                nc.sync.dma_start(gb_sb[:, :, :], d_in[f"gb_{cvn}"][:, :, :])
                Rmax = max(1, 512 // (2 * Ho))
                chunks = [(r0, min(Rmax, Ho - r0)) for r0 in range(0, Ho, Rmax)]
                stats = statp.tile([128, MT, len(chunks), 6], F32, tag="stats")
                for mt in range(MT):
                    for ci, (r0, R) in enumerate(chunks):
                        ps = psump.tile([128, 512], F32, tag="cps")
                        pv = ps[:, 0:2 * R * Ho]
                        n = 0
                        for kt in range(KT):
                            for t in range(T):
                                ky, kx = t // kw + po, t % kw + po
                                if st == 1:
                                    rhs = inbuf[:, kt, :, r0 + ky:r0 + ky + R, kx:kx + Ho]
                                else:
                                    vv = inbuf[:, kt, :, :, :].rearrange(
                                        "p i (h a) (w b) -> p i h a w b", a=2, b=2)
                                    rhs = vv[:, :, r0 + ky // 2:r0 + ky // 2 + R, ky % 2,
                                             kx // 2:kx // 2 + Ho, kx % 2]
                                blk = (t * KT + kt) * MT + mt
                                nc.tensor.matmul(pv, wsb[:, blk, :], rhs,
                                                 start=(n == 0), stop=(n == T * KT - 1))
                                n += 1
                        pv4 = pv.rearrange("p (i r w) -> p i r w", i=2, r=R)
                        nc.scalar.copy(rawbuf[:, mt, :, r0:r0 + R, :], pv4)
                        nc.vector.bn_stats(stats[:, mt, ci, :], pv)
                return bn_reduce(stats, len(chunks), gb_sb, MT, 128)

            def apply_relu(ab, rawv, outbuf, MT, Ho):
                for mt in range(MT):
                    nc.scalar.activation(outbuf[:, mt, :, 1:Ho + 1, 1:Ho + 1],
                                         rawv[:, mt, :, :, :], AF.Relu,
                                         bias=ab[:, 1, mt:mt + 1], scale=ab[:, 0, mt:mt + 1])

            def combine(ab2, raw2v, skipbuf, outbuf, MT, Ho, abd=None, rawdv=None):
                for mt in range(MT):
                    t = bigtmp.tile([128, 2 * Ho * Ho], BF16, tag="combt")
                    tv = t[:, :].rearrange("p (i h w) -> p i h w", i=2, h=Ho)
                    nc.scalar.activation(tv, raw2v[:, mt, :, :, :], AF.Identity,
                                         bias=ab2[:, 1, mt:mt + 1], scale=ab2[:, 0, mt:mt + 1])
                    if abd is not None:
                        sk = bigtmp.tile([128, 2 * Ho * Ho], BF16, tag="combsk")
                        skv = sk[:, :].rearrange("p (i h w) -> p i h w", i=2, h=Ho)
                        nc.scalar.activation(skv, rawdv[:, mt, :, :, :], AF.Identity,
                                             bias=abd[:, 1, mt:mt + 1], scale=abd[:, 0, mt:mt + 1])
                    else:
                        skv = skipbuf[:, mt, :, 1:Ho + 1, 1:Ho + 1]
                    dst = outbuf[:, mt, :, 1:Ho + 1, 1:Ho + 1]
                    nc.vector.tensor_tensor(dst, tv, skv, OP.add)
                    nc.vector.tensor_scalar_max(dst, dst, 0.0)

            # ================= STEM =================
            with tc.tile_pool(name="stemp", bufs=1) as stemp, \
                 tc.tile_pool(name="spsum", bufs=4, space="PSUM") as spsum:
                stack = stemp.tile([48, 115, 115], F32, tag="big")
                wst = stemp.tile([48, 4, 64], F32, tag="wst")
                nc.sync.dma_start(wst[:, :, :], d_in["w_stem"][:, :, :].rearrange("s k m -> k s m"))
                gb_stem = gbp.tile([128, 2, 1], F32, tag="gb")
                nc.sync.dma_start(gb_stem[:, :, :], d_in["gb_stem"][:, :, :])
                raw_st = stemp.tile([64, 2, 112, 112], BF16, tag="rawst")
                nst = 28
                stats_st = statp.tile([64, 1, 2 * nst, 6], F32, tag="statstem")
                nc.gpsimd.memset(stack[:, :, :], 0.0)
                for img in range(BPC):
                    for ph in range(4):
                        nc.sync.dma_start(stack[3 * ph:3 * ph + 3, 2:114, 2:114],
                                          d_in["x"][img, ph, :, :, :])
                    for isx in range(1, 4):
                        nc.sync.dma_start(stack[12 * isx:12 * isx + 12, :, 0:115 - isx],
                                          stack[0:12, :, isx:115])
                    for ci in range(nst):
                        r0 = ci * 4
                        ps = spsum.tile([128, 448], F32, tag="cps")
                        for syi in range(4):
                            nc.tensor.matmul(ps[0:64, :], wst[:, syi, :],
                                             stack[:, r0 + syi:r0 + syi + 4, 0:112],
                                             start=(syi == 0), stop=(syi == 3))
                        nc.scalar.copy(raw_st[:, img, r0:r0 + 4, :],
                                       ps[0:64, :].rearrange("p (r w) -> p r w", r=4))
                        nc.vector.bn_stats(stats_st[:, 0, img * nst + ci, :], ps[0:64, :])
                ab_st = bn_reduce(stats_st, 2 * nst, gb_stem, 1, 64)
                pp = stemp.tile([64, 2, 114, 114], BF16, tag="big")
                nc.gpsimd.memset(pp[:, :, :, :], 0.0)
                nc.scalar.activation(pp[:, :, 1:113, 1:113], raw_st[:, :, :, :], AF.Relu,
                                     bias=ab_st[0:64, 1, 0:1], scale=ab_st[0:64, 0, 0:1])
                ppv = pp[:, :, :, :].rearrange("p i r (w a) -> p i r w a", a=2)
                m1 = stemp.tile([64, 2, 114, 56], BF16, tag="rawst")
                nc.vector.tensor_tensor(m1[:, :, :, :], ppv[:, :, :, 0:56, 0],
                                        ppv[:, :, :, 0:56, 1], OP.max)
                nc.vector.tensor_tensor(m1[:, :, :, :], m1[:, :, :, :],
                                        ppv[:, :, :, 1:57, 0], OP.max)
                m1v = m1[:, :, :, :].rearrange("p i (r a) w -> p i r a w", a=2)
                out1 = padA[1][0:64, 0, :, 1:57, 1:57]
                nc.vector.tensor_tensor(out1, m1v[:, :, 0:56, 0, :], m1v[:, :, 0:56, 1, :], OP.max)
                nc.vector.tensor_tensor(out1, out1, m1v[:, :, 1:57, 0, :], OP.max)

            # ================= residual layers =================
            with tc.tile_pool(name="wpool", bufs=2) as wpool, \
                 tc.tile_pool(name="wpool4", bufs=1) as wpool4, \
                 tc.tile_pool(name="bigtmp", bufs=2) as bigtmp, \
                 tc.tile_pool(name="cpsum", bufs=6, space="PSUM") as cpsum:
                for (li, bi, has_ds) in blocks_meta:
                    Ho, MT = RES[li], CT_OF[li]
                    wp = wpool4 if li == 4 else wpool
                    inb = padA[li] if bi > 0 else padA[max(li - 1, 1)]
                    ra, rb = raw_view(rawA, li), raw_view(rawB, li)
                    rd = raw_view(rawD, li) if has_ds else None
                    ab1 = conv(f"l{li}b{bi}c1", inb, ra, wp, cpsum)
                    apply_relu(ab1, ra, padB[li], MT, Ho)
                    ab2 = conv(f"l{li}b{bi}c2", padB[li], rb, wp, cpsum)
                    if has_ds:
                        abd = conv(f"l{li}b{bi}ds", inb, rd, wp, cpsum)
                        combine(ab2, rb, None, padA[li], MT, Ho, abd=abd, rawdv=rd)
                    else:
                        combine(ab2, rb, inb, padA[li], MT, Ho)

            # ================= feat AG + fc1 + gx =================
            featb = dram.tile([BPC, 25088], BF16, tag="featin")
            for ct in range(4):
                ov = o_feat[:, ct * 6272:(ct + 1) * 6272].rearrange("i (c h w) -> c i h w", c=128, h=7)
                for img in range(BPC):
                    fv = featb[img, ct * 6272:(ct + 1) * 6272].rearrange("(c h w) -> c h w", c=128, h=7)
                    nc.sync.dma_start(fv, padA[4][:, ct, img, 1:8, 1:8])
                    nc.gpsimd.dma_start(ov[:, img, :, :], padA[4][:, ct, img, 1:8, 1:8])
            featg = dram.tile([8, BPC, 25088], BF16, tag="featout")
            nc.gpsimd.collective_compute("AllGather", OP.bypass, ins=[featb.opt()],
                                         outs=[featg.opt()], replica_groups=RG)
            gxS = nc.alloc_sbuf_tensor("gxS", [16, 512], F32)
            with tc.tile_pool(name="fc1p", bufs=1) as fc1p, \
                 tc.tile_pool(name="fpsum", bufs=1, space="PSUM") as fpsum:
                featT = fc1p.tile([128, 196, 16], BF16, tag="featT")
                fgv = featg[:, :, :].rearrange("r i (t p) -> p t r i", p=128)
                ftv = featT[:, :, :].rearrange("p t (r i) -> p t r i", r=8)
                for r in range(8):
                    for i in range(BPC):
                        nc.sync.dma_start(ftv[:, :, r, i], fgv[:, :, r, i])
                w1sb = fc1p.tile([128, 196, 128], BF16, tag="w1sb")
                nc.sync.dma_start(w1sb[:, :, :], d_in["w1sT"][:, :, :].rearrange("t p o -> p t o"))
                onesb = fc1p.tile([1, 128], BF16, tag="onesb")
                nc.sync.dma_start(onesb[:, :], d_in["ones_bf"][:, :])
                b1sb = fc1p.tile([1, 128], BF16, tag="b1sb")
                nc.sync.dma_start(b1sb[:, :], d_in["b1row"][:, :])
                psx = fpsum.tile([16, 128], F32, tag="psx")
                for t in range(196):
                    nc.tensor.matmul(psx[:, :], featT[:, t, :], w1sb[:, t, :],
                                     start=(t == 0), stop=False)
                nc.tensor.matmul(psx[:, :], onesb[0:1, 0:16], b1sb[:, :], start=False, stop=True)
                xinS = fc1p.tile([16, 128], F32, tag="xinS")
                nc.scalar.activation(xinS[:, :], psx[:, :], AF.Relu)
                nc.sync.dma_start(o_xin[:, :], xinS[:, :])
                idn = fc1p.tile([16, 16], F32, tag="idn")
                nc.sync.dma_start(idn[:, :], d_in["ident"][:, :])
                pst = fpsum.tile([128, 16], F32, tag="pst")
                nc.tensor.transpose(pst[:, :], xinS[:, :], idn[:, :])
                xinT = fc1p.tile([128, 16], F32, tag="xinT")
                nc.vector.tensor_copy(xinT[:, :], pst[:, :])
                xb = dram.tile([128, 16], F32, tag="xinb")
                nc.sync.dma_start(xb[:, :], xinT[:, :])
                xg = dram.tile([8, 128, 16], F32, tag="xing")
                nc.gpsimd.collective_compute("AllGather", OP.bypass, ins=[xb.opt()],
                                             outs=[xg.opt()], replica_groups=RG)
                xinTg = fc1p.tile([128, 8, 16], F32, tag="xinTg")
                nc.sync.dma_start(xinTg[:, :, :], xg[:, :, :].rearrange("r p i -> p r i"))
                wih = fc1p.tile([128, 8, 512], F32, tag="wih")
                nc.sync.dma_start(wih[:, :, :], d_in["wihT"][:, :, :].rearrange("k p o -> p k o"))
                onesf = fc1p.tile([1, 16], F32, tag="onesf")
                nc.sync.dma_start(onesf[:, :], d_in["ones_f"][:, :])
                brow = fc1p.tile([1, 512], F32, tag="brow")
                nc.sync.dma_start(brow[:, :], d_in["brow"][:, :])
                psg = fpsum.tile([16, 512], F32, tag="psg")
                for k in range(8):
                    nc.tensor.matmul(psg[:, :], xinTg[:, k, :], wih[:, k, :],
                                     start=(k == 0), stop=False)
                nc.tensor.matmul(psg[:, :], onesf[:, :], brow[:, :], start=False, stop=True)
                nc.scalar.copy(gxS[:, :], psg[:, :])
                nc.sync.dma_start(o_gx[:, :], gxS[:, :])

            # ================= LSTM =================
            with tc.tile_pool(name="latep", bufs=1) as latep, \
                 tc.tile_pool(name="lp", bufs=4) as lp, \
                 tc.tile_pool(name="ld", bufs=4, space="DRAM") as ld, \
                 tc.tile_pool(name="lps", bufs=2, space="PSUM") as lps:
                hsT = latep.tile([128, 8, B, DEC], BF16, tag="hsT")
                whh = latep.tile([128, 8, 512], F32, tag="whh")
                nc.sync.dma_start(whh[:, :, :], d_in["whhT"][:, :, :].rearrange("k p o -> p k o"))
                idn2 = latep.tile([16, 16], F32, tag="idn2")
                nc.sync.dma_start(idn2[:, :], d_in["ident"][:, :])
                hT = latep.tile([128, 8, 16], F32, tag="hT")
                cS = latep.tile([16, 128], F32, tag="cS")
                nc.vector.memset(hT[:, :, :], 0.0)
                nc.vector.memset(cS[:, :], 0.0)
                for step in range(DEC):
                    ps = lps.tile([16, 512], F32, tag="g")
                    for k in range(8):
                        nc.tensor.matmul(ps[:, :], hT[:, k, :], whh[:, k, :],
                                         start=(k == 0), stop=(k == 7))
                    gS = lp.tile([16, 512], F32, tag="gS")
                    nc.vector.tensor_tensor(gS[:, :], ps[:, :], gxS[:, :], OP.add)
                    sig = lp.tile([16, 384], F32, tag="sig")
                    nc.scalar.activation(sig[:, :], gS[:, 0:384], AF.Sigmoid)
                    tg = lp.tile([16, 128], F32, tag="tg")
                    nc.scalar.activation(tg[:, :], gS[:, 384:512], AF.Tanh)
                    t1 = lp.tile([16, 128], F32, tag="t1")
                    nc.vector.tensor_tensor(t1[:, :], sig[:, 0:128], tg[:, :], OP.mult)
                    nc.vector.tensor_tensor(cS[:, :], sig[:, 128:256], cS[:, :], OP.mult)
                    nc.vector.tensor_tensor(cS[:, :], cS[:, :], t1[:, :], OP.add)
                    tcv = lp.tile([16, 128], F32, tag="tc")
                    nc.scalar.activation(tcv[:, :], cS[:, :], AF.Tanh)
                    hS = lp.tile([16, 128], F32, tag="hS")
                    nc.vector.tensor_tensor(hS[:, :], sig[:, 256:384], tcv[:, :], OP.mult)
                    pst2 = lps.tile([128, 16], F32, tag="ptr")
                    nc.tensor.transpose(pst2[:, :], hS[:, :], idn2[:, :])
                    hTl = lp.tile([128, 16], F32, tag="hTl")
                    nc.vector.tensor_copy(hTl[:, :], pst2[:, :])
                    bin_ = ld.tile([128, 16], F32, tag="hin")
                    nc.sync.dma_start(bin_[:, :], hTl[:, :])
                    bout = ld.tile([8, 128, 16], F32, tag="hout")
                    nc.gpsimd.collective_compute("AllGather", OP.bypass, ins=[bin_.opt()],
                                                 outs=[bout.opt()], replica_groups=RG)
                    nc.sync.dma_start(hT[:, :, :], bout[:, :, :].rearrange("k p i -> p k i"))
                    nc.vector.tensor_copy(hsT[:, :, :, step], hT[:, :, :])
                nc.sync.dma_start(o_hT[:, :], hT[:, :, :].rearrange("p k i -> p (k i)"))

                # ================= heads =================
                with tc.tile_pool(name="hp", bufs=3) as hp, \
                     tc.tile_pool(name="hps", bufs=3, space="PSUM") as hps:
                    whc = hp.tile([128, 8, 198], BF16, tag="whc")
                    nc.sync.dma_start(whc[:, :, :], d_in["whcT"][:, :, :].rearrange("k p o -> p k o"))
                    onesb2 = hp.tile([1, 128], BF16, tag="ones2")
                    nc.sync.dma_start(onesb2[:, :], d_in["ones_bf"][:, :])
                    bh = hp.tile([1, 198], BF16, tag="bh")
                    nc.sync.dma_start(bh[:, :], d_in["bhrow"][:, :])
                    hsTv = hsT[:, :, :, :].rearrange("p k i s -> p k (i s)")
                    rows_total = B * DEC
                    for mt in range(_ceil(rows_total, 128)):
                        r0 = mt * 128
                        rn = min(128, rows_total - r0)
                        ps3 = hps.tile([128, 198], F32, tag="hps")
                        for k in range(8):
                            nc.tensor.matmul(ps3[0:rn, :], hsTv[:, k, r0:r0 + rn], whc[:, k, :],
                                             start=(k == 0), stop=False)
                        nc.tensor.matmul(ps3[0:rn, :], onesb2[:, 0:rn], bh[:, :],
                                         start=False, stop=True)
                        tab = hp.tile([128, 198], F32, tag="tab")
                        nc.scalar.copy(tab[0:rn, :], ps3[0:rn, :])
                        nc.sync.dma_start(o_table[r0:r0 + rn, :], tab[0:rn, 0:197])
                        nc.sync.dma_start(o_values[r0:r0 + rn, :], tab[0:rn, 197:198])

    nc.finalize()
    return nc


# ---------------------------------------------------------------- entry

def kernel(**inputs):
    from concourse.bass_utils import run_bass_kernel_spmd
    convs, blocks_meta, in_maps = _host_prep(inputs)
    if "nc" not in _cached:
        _cached["nc"] = _build(convs, blocks_meta)
    res = run_bass_kernel_spmd(_cached["nc"], in_maps, core_ids=list(range(N_CORES)),
                               trace=False)
    _cached["last"] = res
    r0 = res.results[0]
    table = r0["table"].reshape(B, DEC, ACTN)
    values = r0["values"].reshape(B, DEC, 1)
    return table, values
